# revision 5
# baseline (speedup 1.0000x reference)
"""Trainium2 Bass kernel for nn_MultiHeadAttn (B=2, S=2048, D=1024, H=16).

Returns (out, att) like the reference:
    qp = (q @ Wq.T).reshape(B, H, S, depth)   # raw reshape: head h <-> rows
    kp, vp likewise                            # [h*128,(h+1)*128) of the S x D
    qk = qp @ kp.T per (b, h)                  # projection matrix
    att = softmax(sqrt(qk) + msk * -1e9)       # msk is all-zeros per spec
    out = (att @ vp).reshape(B, S, D) @ Wfc.T

Sharding: 32 independent (b, h) pieces; core c owns batch c//4, heads
[4*(c%4), 4*(c%4)+4)  (rows [512*(c%4), +512) of that batch).

Key layout trick: the raw reshape means head h's [S, depth] view of the
projection interleaves: element (s, d) = proj[h*128 + s//16, (s%16)*64 + d].
We work in "s_lo-major" permuted order q~ = s_lo*128 + r internally; the
permutation is undone for free inside strided APs of the ACT ln pass and
the att/out DMAs.

Per piece pipeline (all matmuls fp16 with f32 PSUM accumulate):
  proj q/k: W-stationary -> projT tiles [m, r] -> lane-shift DVE copies into
            QhT/KhT [64(d), head*2048 + s_lo*128 + r] fp16
  proj v:   v-stationary -> proj_v [r, m] -> expand-DMA into vp [k, d] fp16
  qk:       QhT x KhT -> PSUM [q~, k~]
  softmax:  ACT ln (unpermutes k~ -> k), ACT exp(0.5 t) = sqrt, ACT
            exp(r - 40) + row-sum accum; DVE reciprocal + 2 normalizes
            (f32 for att output, fp16 for the transpose path)
  ctx:      xbar-transpose att fp16 -> att_T [k, q~]; vp-stationary matmul
            -> ctx.T [d, q~]; lane-shift copies -> ctx_flat.T [j, r] fp16
  fc:       ctx_flat.T x WfcT -> out rows [r, m]
"""

import sys

sys.path.insert(0, "/opt/trn_rl_repo")

import numpy as np
import ml_dtypes

import concourse.bass as bass
import concourse.tile as tile
from concourse import bacc, mybir
from concourse import bass_utils

F32 = mybir.dt.float32
F16 = mybir.dt.float16
AF = mybir.ActivationFunctionType

B, S, D, H = 2, 2048, 1024, 16
DEPTH = D // H          # 64
NC = 8                  # cores
HPC = H // (NC // B)    # heads per core = 4
ROWS = HPC * 128        # projection rows per core = 512
EXP_BIAS = -40.0        # logits ~ sqrt(qk) in [30, 50]; keeps e^x in range

# dma_start_transpose 3D out-AP ordering; set from hardware experiment.
TP_ORDER = "A"  # "A": out [p, kc, q]   "B": out [kc, p, q]


def _build():
    nc = bacc.Bacc(
        "TRN2",
        target_bir_lowering=False,
        debug=False,
        enable_asserts=False,
        num_devices=NC,
    )

    qT = nc.dram_tensor("qT", [D, ROWS], F16, kind="ExternalInput")
    kT = nc.dram_tensor("kT", [D, ROWS], F16, kind="ExternalInput")
    vT = nc.dram_tensor("vT", [D, ROWS], F16, kind="ExternalInput")
    WqT = nc.dram_tensor("WqT", [D, D], F16, kind="ExternalInput")
    WkT = nc.dram_tensor("WkT", [D, D], F16, kind="ExternalInput")
    WvT = nc.dram_tensor("WvT", [D, D], F16, kind="ExternalInput")
    WfcT = nc.dram_tensor("WfcT", [D, D], F16, kind="ExternalInput")
    att_o = nc.dram_tensor("att_o", [HPC, S, S], F32, kind="ExternalOutput")
    out_o = nc.dram_tensor("out_o", [ROWS, D], F32, kind="ExternalOutput")

    with tile.TileContext(nc) as tc:
        _emit(nc, tc, qT, kT, vT, WqT, WkT, WvT, WfcT, att_o, out_o)
    nc.finalize()
    return nc


def _emit(nc, tc, qT, kT, vT, WqT, WkT, WvT, WfcT, att_o, out_o):
    # ---------------- persistent SBUF tensors ----------------
    persist_cm = tc.tile_pool(name="persist", bufs=1)
    persist = persist_cm.__enter__()
    # QhT/KhT: [64(d), hpc*2048 + s_lo*128 + r] fp16
    QhT = persist.tile([64, HPC * S], F16, tag="QhT")
    KhT = persist.tile([64, HPC * S], F16, tag="KhT")
    # vp: per piece [128(k within chunk), kc*64 + d] fp16, pieces side by side
    vp = persist.tile([128, HPC * 16 * DEPTH], F16, tag="vp")
    # att_T accumulator for one piece: [128(k in chunk), kc*2048 + q~] fp16
    attT = persist.tile([128, 16 * S], F16, tag="attT")
    # fc weights
    wfc = persist.tile([128, 8 * D], F16, tag="wfc")
    bias_t = persist.tile([128, 1], F32, tag="bias")
    nc.gpsimd.memset(bias_t[:], EXP_BIAS)
    for j in range(8):
        nc.sync.dma_start(wfc[:, j * D:(j + 1) * D], WfcT[j * 128:(j + 1) * 128, :])

    # ---------------- phase A: projections q, k (W-stationary) -------------
    with (
        tc.tile_pool(name="projw", bufs=1) as wpool,
        tc.tile_pool(name="projx", bufs=1) as xpool,
        tc.tile_pool(name="projps", bufs=2, space="PSUM") as ppsum,
    ):
        for WT_d, xT_d, dest in ((WqT, qT, QhT), (WkT, kT, KhT)):
            wts = []
            xts = []
            for j in range(8):
                wt = wpool.tile([128, D], F16, tag=f"w{j}")
                nc.sync.dma_start(wt[:], WT_d[j * 128:(j + 1) * 128, :])
                wts.append(wt)
                xt = xpool.tile([128, ROWS], F16, tag=f"x{j}")
                nc.sync.dma_start(xt[:], xT_d[j * 128:(j + 1) * 128, :])
                xts.append(xt)
            for mc in range(8):
                ps = ppsum.tile([128, ROWS], F32, tag="pp")
                for j in range(8):
                    nc.tensor.matmul(
                        ps[:], wts[j][:, mc * 128:(mc + 1) * 128], xts[j][:],
                        start=(j == 0), stop=(j == 7),
                    )
                # rows [par*64, +64) of ps hold m = s_lo*64 + d for
                # s_lo = 2*mc + par; scatter to dest[d, h*2048 + s_lo*128 + r]
                for par in range(2):
                    s_lo = 2 * mc + par
                    src = ps[par * 64:(par + 1) * 64, :].rearrange(
                        "p (h r) -> p h r", h=HPC)
                    dst = dest[:, :].rearrange(
                        "p (h sl r) -> p h sl r", h=HPC, sl=16)[:, :, s_lo, :]
                    nc.vector.tensor_copy(dst, src)

        # ---------------- phase B: projection v (v-stationary) ------------
        wv = []
        for j in range(8):
            t = wpool.tile([128, D], F16, tag=f"wv{j}")
            nc.sync.dma_start(t[:], WvT[j * 128:(j + 1) * 128, :])
            wv.append(t)
        vts = []
        for j in range(8):
            t = xpool.tile([128, ROWS], F16, tag=f"v{j}")
            nc.sync.dma_start(t[:], vT[j * 128:(j + 1) * 128, :])
            vts.append(t)
        projv = persist.tile([128, HPC * D], F16, tag="projv")
        for p in range(HPC):
            for mh in range(2):
                ps = ppsum.tile([128, 512], F32, tag="pp")
                for j in range(8):
                    nc.tensor.matmul(
                        ps[:], vts[j][:, p * 128:(p + 1) * 128],
                        wv[j][:, mh * 512:(mh + 1) * 512],
                        start=(j == 0), stop=(j == 7),
                    )
                nc.vector.tensor_copy(
                    projv[:, p * D + mh * 512: p * D + (mh + 1) * 512], ps[:])
        # expand: vp[p][kc*64 + d] at partition i <- projv[8kc + i//16,
        #         p*D + (i%16)*64 + d]
        for p in range(HPC):
            for kc in range(16):
                nc.sync.dma_start(
                    vp[:, p * 1024 + kc * 64: p * 1024 + (kc + 1) * 64],
                    projv[8 * kc:8 * kc + 8, p * D:(p + 1) * D].rearrange(
                        "p (sl d) -> p sl d", sl=16),
                )

    # ---------------- phase C: attention per piece ----------------
    with (
        tc.tile_pool(name="qkps", bufs=1, space="PSUM") as qkpsum,
        tc.tile_pool(name="ctxps", bufs=2, space="PSUM") as ctxpsum,
        tc.tile_pool(name="outps", bufs=1, space="PSUM") as outpsum,
        tc.tile_pool(name="work", bufs=4) as work,
        tc.tile_pool(name="workh", bufs=2) as workh,
        tc.tile_pool(name="small", bufs=4) as small,
    ):
        for p in range(HPC):
            for qc in range(16):
                ps_qk = qkpsum.tile([128, S], F32, tag="qk")
                lhs = QhT[:, p * S + qc * 128: p * S + (qc + 1) * 128]
                for kn in range(4):
                    nc.tensor.matmul(
                        ps_qk[:, kn * 512:(kn + 1) * 512], lhs,
                        KhT[:, p * S + kn * 512: p * S + (kn + 1) * 512],
                        start=True, stop=True,
                    )
                # pass 1: t = ln(qk), unpermuting k~ = (sl, r) -> k = (r, sl)
                t_ln = work.tile([128, S], F32, tag="wf")
                nc.scalar.activation(
                    t_ln[:].rearrange("p (r sl) -> p r sl", sl=16),
                    ps_qk[:].rearrange("p (sl r) -> p r sl", sl=16),
                    AF.Ln,
                )
                # pass 2: r = exp(0.5 * t) = sqrt(qk)
                r_t = work.tile([128, S], F32, tag="wf")
                nc.scalar.activation(r_t[:], t_ln[:], AF.Exp, scale=0.5)
                # pass 3: e = exp(r - 40), row sums into dsum
                e_t = work.tile([128, S], F32, tag="wf")
                dsum = small.tile([128, 1], F32, tag="ds")
                nc.scalar.activation(
                    e_t[:], r_t[:], AF.Exp, bias=bias_t[:], accum_out=dsum[:])
                rec = small.tile([128, 1], F32, tag="rc")
                nc.vector.reciprocal(rec[:], dsum[:])
                att_f = work.tile([128, S], F32, tag="wf")
                nc.vector.tensor_scalar_mul(att_f[:], e_t[:], rec[:])
                nc.sync.dma_start(
                    att_o[p].rearrange("(r sl) k -> sl r k", sl=16)[qc],
                    att_f[:],
                )
                att_h = workh.tile([128, S], F16, tag="wh")
                nc.vector.tensor_scalar_mul(att_h[:], e_t[:], rec[:])
                if TP_ORDER == "A":
                    dst = attT[:, :].rearrange(
                        "p (kc q) -> p kc q", kc=16)[:, :, qc * 128:(qc + 1) * 128]
                else:
                    dst = attT[:, :].rearrange(
                        "p (kc q) -> kc p q", kc=16)[:, :, qc * 128:(qc + 1) * 128]
                nc.sync.dma_start_transpose(dst, att_h[:])

            # ctx.T = sum_k vp[k, d] * att_T[k, q~]  -> [64(d), q~]
            ctxT = workh.tile([128, 8 * 128], F16, tag="ctxT")
            for qn in range(4):
                ps_ctx = ctxpsum.tile([64, 512], F32, tag="ctx")
                for kc in range(16):
                    nc.tensor.matmul(
                        ps_ctx[:],
                        vp[:, p * 1024 + kc * 64: p * 1024 + (kc + 1) * 64],
                        attT[:, kc * S + qn * 512: kc * S + (qn + 1) * 512],
                        start=(kc == 0), stop=(kc == 15),
                    )
                # scatter into ctx_flat.T [j = s_lo*64 + d, r] fp16
                for i in range(4):
                    s_lo = 4 * qn + i
                    nc.vector.tensor_copy(
                        ctxT[(s_lo % 2) * 64:(s_lo % 2) * 64 + 64,
                             (s_lo // 2) * 128:(s_lo // 2) * 128 + 128],
                        ps_ctx[:, i * 128:(i + 1) * 128],
                    )
            ps_out = outpsum.tile([128, D], F32, tag="out")
            for j in range(8):
                for mn in range(2):
                    nc.tensor.matmul(
                        ps_out[:, mn * 512:(mn + 1) * 512],
                        ctxT[:, j * 128:(j + 1) * 128],
                        wfc[:, j * D + mn * 512: j * D + (mn + 1) * 512],
                        start=(j == 0), stop=(j == 7),
                    )
            out_sb = work.tile([128, D], F32, tag="osb")
            nc.vector.tensor_copy(out_sb[:], ps_out[:])
            nc.sync.dma_start(out_o[p * 128:(p + 1) * 128, :], out_sb[:])

    persist_cm.__exit__(None, None, None)


_NC_CACHE = None


def _get_nc():
    global _NC_CACHE
    if _NC_CACHE is None:
        _NC_CACHE = _build()
    return _NC_CACHE


def _prep_inputs(q, k, v, Wq, Wk, Wv, Wfc):
    f16 = ml_dtypes.float16 if hasattr(ml_dtypes, "float16") else np.float16
    WqT = np.ascontiguousarray(Wq.T).astype(f16)
    WkT = np.ascontiguousarray(Wk.T).astype(f16)
    WvT = np.ascontiguousarray(Wv.T).astype(f16)
    WfcT = np.ascontiguousarray(Wfc.T).astype(f16)
    in_maps = []
    for c in range(NC):
        b, g = c // (NC // B), c % (NC // B)
        rows = slice(ROWS * g, ROWS * (g + 1))
        in_maps.append({
            "qT": np.ascontiguousarray(q[b, rows, :].T).astype(f16),
            "kT": np.ascontiguousarray(k[b, rows, :].T).astype(f16),
            "vT": np.ascontiguousarray(v[b, rows, :].T).astype(f16),
            "WqT": WqT, "WkT": WkT, "WvT": WvT, "WfcT": WfcT,
        })
    return in_maps


def kernel(v, k, q, msk, Wq, Wk, Wv, Wfc, _trace=False, _trace_kwargs=None):
    q = np.asarray(q, np.float32)
    k = np.asarray(k, np.float32)
    v = np.asarray(v, np.float32)
    # msk is all-zeros per the problem spec (fill: zeros); msk * -1e9 == 0.
    nc = _get_nc()
    in_maps = _prep_inputs(q, k, v, np.asarray(Wq), np.asarray(Wk),
                           np.asarray(Wv), np.asarray(Wfc))
    kw = dict(_trace_kwargs or {})
    res = bass_utils.run_bass_kernel_spmd(
        nc, in_maps, core_ids=list(range(NC)), trace=_trace, **kw)
    att = np.empty((B, H, S, S), np.float32)
    out = np.empty((B, S, D), np.float32)
    for c in range(NC):
        b, g = c // (NC // B), c % (NC // B)
        r = res.results[c]
        att[b, HPC * g: HPC * (g + 1)] = r["att_o"]
        out[b, ROWS * g: ROWS * (g + 1), :] = r["out_o"]
    kernel.last_results = res
    return out, att


# revision 17
# speedup vs baseline: 24584.8289x; 24584.8289x over previous
"""Trainium2 Bass kernel for nn_MultiHeadAttn (B=2, S=2048, D=1024, H=16).

Returns (out, att) like the reference:
    qp = (q @ Wq.T).reshape(B, H, S, depth)   # raw reshape: head h <-> rows
    kp, vp likewise                            # [h*128,(h+1)*128) of the S x D
    qk = qp @ kp.T per (b, h)                  # projection matrix
    att = softmax(sqrt(qk) + msk * -1e9)       # msk is all-zeros per spec
    out = (att @ vp).reshape(B, S, D) @ Wfc.T

Sharding: 32 independent (b, h) pieces; core c owns batch c//4, heads
[4*(c%4), 4*(c%4)+4)  (rows [512*(c%4), +512) of that batch).

Key layout trick: the raw reshape means head h's [S, depth] view of the
projection interleaves: element (s, d) = proj[h*128 + s//16, (s%16)*64 + d].
We work in "s_lo-major" permuted order q~ = s_lo*128 + r internally; the
permutation is undone for free inside strided APs of the ACT ln pass and
the att/out DMAs.

Per piece pipeline (all matmuls fp16 with f32 PSUM accumulate):
  proj q/k: W-stationary -> projT tiles [m, r] -> lane-shift DVE copies into
            QhT/KhT [64(d), head*2048 + s_lo*128 + r] fp16
  proj v:   v-stationary -> proj_v [r, m] -> expand-DMA into vp [k, d] fp16
  qk:       QhT x KhT -> PSUM [q~, k~]
  softmax:  ACT ln (unpermutes k~ -> k), ACT exp(0.5 t) = sqrt, ACT
            exp(r - 40) + row-sum accum; DVE reciprocal + 2 normalizes
            (f32 for att output, fp16 for the transpose path)
  ctx:      xbar-transpose att fp16 -> att_T [k, q~]; vp-stationary matmul
            -> ctx.T [d, q~]; lane-shift copies -> ctx_flat.T [j, r] fp16
  fc:       ctx_flat.T x WfcT -> out rows [r, m]
"""

import sys

sys.path.insert(0, "/opt/trn_rl_repo")

import numpy as np
import ml_dtypes

import concourse.bass as bass
import concourse.tile as tile
from concourse import bacc, mybir
from concourse import bass_utils

F32 = mybir.dt.float32
F16 = mybir.dt.float16
AF = mybir.ActivationFunctionType

# Our only transcendentals are Ln and Exp. The default table picker chooses
# the first set containing each function, which alternates between the
# `natural_log` and `exp_and_others` sets -> one ~1.3us ACT_TABLE_LOAD per
# activation. Strip Ln/Exp from every set except the combined
# `natural_log_exp_and_others` so a single table load is hoisted.
_orig_get_tables = bacc.get_activation_tables


def _patched_get_tables(arch):
    tables = _orig_get_tables(arch)
    for name, funcs in tables.items():
        if name != "natural_log_exp_and_others":
            funcs.discard(AF.Ln)
            funcs.discard(AF.Exp)
    return tables


bacc.get_activation_tables = _patched_get_tables

B, S, D, H = 2, 2048, 1024, 16
DEPTH = D // H          # 64
NC = 8                  # cores
HPC = H // (NC // B)    # heads per core = 4
ROWS = HPC * 128        # projection rows per core = 512
EXP_BIAS = -40.0        # logits ~ sqrt(qk) in [30, 50]; keeps e^x in range

# dma_start_transpose 3D out-AP ordering; set from hardware experiment.
TP_ORDER = "A"  # "A": out [p, kc, q]   "B": out [kc, p, q]


def _build():
    nc = bacc.Bacc(
        "TRN2",
        target_bir_lowering=False,
        debug=False,
        enable_asserts=False,
        num_devices=NC,
    )

    qT = nc.dram_tensor("qT", [D, ROWS], F16, kind="ExternalInput")
    kT = nc.dram_tensor("kT", [D, ROWS], F16, kind="ExternalInput")
    vT = nc.dram_tensor("vT", [D, ROWS], F16, kind="ExternalInput")
    WqT = nc.dram_tensor("WqT", [D, D], F16, kind="ExternalInput")
    WkT = nc.dram_tensor("WkT", [D, D], F16, kind="ExternalInput")
    WvT = nc.dram_tensor("WvT", [D, D], F16, kind="ExternalInput")
    WfcT = nc.dram_tensor("WfcT", [D, D], F16, kind="ExternalInput")
    att_o = nc.dram_tensor("att_o", [HPC, S, S], F32, kind="ExternalOutput")
    out_o = nc.dram_tensor("out_o", [ROWS, D], F32, kind="ExternalOutput")

    with tile.TileContext(nc) as tc:
        _emit(nc, tc, qT, kT, vT, WqT, WkT, WvT, WfcT, att_o, out_o)
    nc.finalize()
    return nc


def _emit(nc, tc, qT, kT, vT, WqT, WkT, WvT, WfcT, att_o, out_o):
    # ---------------- persistent SBUF tensors ----------------
    persist_cm = tc.tile_pool(name="persist", bufs=1)
    persist = persist_cm.__enter__()
    # QhT/KhT: [64(d), hpc*2048 + s_lo*128 + r] fp16
    QhT = persist.tile([64, HPC * S], F16, tag="QhT")
    KhT = persist.tile([64, HPC * S], F16, tag="KhT")
    # vp: per piece [128(k within chunk), kc*64 + d] fp16, pieces side by side
    vp = persist.tile([128, HPC * 16 * DEPTH], F16, tag="vp")
    # att_T accumulators, one per q~ 512-group: [128(k in chunk),
    # kc*512 + q~local] fp16. Split so next piece's transposes only WAR
    # against the one ctx group that reads the same quarter.
    attT = []
    for g in range(4):
        attT_g = persist.tile([128, 16 * 512], F16, tag=f"attT{g}")
        attT.append(attT_g)
    # fc weights
    wfc = persist.tile([128, 8 * D], F16, tag="wfc")
    bias_t = persist.tile([128, 1], F32, tag="bias")
    nc.gpsimd.memset(bias_t[:], EXP_BIAS)
    for j in range(8):
        nc.sync.dma_start(wfc[:, j * D:(j + 1) * D], WfcT[j * 128:(j + 1) * 128, :])

    # ---------------- phase A: projections q, k (W-stationary) -------------
    with (
        tc.tile_pool(name="projw", bufs=1) as wpool,
        tc.tile_pool(name="projx", bufs=1) as xpool,
        tc.tile_pool(name="projps", bufs=2, space="PSUM") as ppsum,
    ):
        for WT_d, xT_d, dest in ((WqT, qT, QhT), (WkT, kT, KhT)):
            wts = []
            xts = []
            for j in range(8):
                wt = wpool.tile([128, D], F16, tag=f"w{j}")
                nc.sync.dma_start(wt[:], WT_d[j * 128:(j + 1) * 128, :])
                wts.append(wt)
                xt = xpool.tile([128, ROWS], F16, tag=f"x{j}")
                nc.sync.dma_start(xt[:], xT_d[j * 128:(j + 1) * 128, :])
                xts.append(xt)
            for mc in range(8):
                ps = ppsum.tile([128, ROWS], F32, tag="pp")
                for j in range(8):
                    nc.tensor.matmul(
                        ps[:], wts[j][:, mc * 128:(mc + 1) * 128], xts[j][:],
                        start=(j == 0), stop=(j == 7),
                    )
                # rows [par*64, +64) of ps hold m = s_lo*64 + d for
                # s_lo = 2*mc + par; scatter to dest[d, h*2048 + s_lo*128 + r]
                for par in range(2):
                    s_lo = 2 * mc + par
                    src = ps[par * 64:(par + 1) * 64, :].rearrange(
                        "p (h r) -> p h r", h=HPC)
                    dst = dest[:, :].rearrange(
                        "p (h sl r) -> p h sl r", h=HPC, sl=16)[:, :, s_lo, :]
                    nc.vector.tensor_copy(dst, src)

        # ---------------- phase B: projection v (v-stationary) ------------
        wv = []
        for j in range(8):
            t = wpool.tile([128, D], F16, tag=f"wv{j}")
            nc.sync.dma_start(t[:], WvT[j * 128:(j + 1) * 128, :])
            wv.append(t)
        vts = []
        for j in range(8):
            t = xpool.tile([128, ROWS], F16, tag=f"v{j}")
            nc.sync.dma_start(t[:], vT[j * 128:(j + 1) * 128, :])
            vts.append(t)
        projv = persist.tile([128, HPC * D], F16, tag="projv")
        for p in range(HPC):
            for mh in range(2):
                ps = ppsum.tile([128, 512], F32, tag="pp")
                for j in range(8):
                    nc.tensor.matmul(
                        ps[:], vts[j][:, p * 128:(p + 1) * 128],
                        wv[j][:, mh * 512:(mh + 1) * 512],
                        start=(j == 0), stop=(j == 7),
                    )
                nc.vector.tensor_copy(
                    projv[:, p * D + mh * 512: p * D + (mh + 1) * 512], ps[:])
        # expand: vp[p][kc*64 + d] at partition i <- projv[8kc + i//16,
        #         p*D + (i%16)*64 + d]
        for p in range(HPC):
            for kc in range(16):
                nc.sync.dma_start(
                    vp[:, p * 1024 + kc * 64: p * 1024 + (kc + 1) * 64],
                    projv[8 * kc:8 * kc + 8, p * D:(p + 1) * D].rearrange(
                        "p (sl d) -> p sl d", sl=16),
                )

    # ---------------- phase C: attention per piece ----------------
    # Full-chunk pipeline. The ln pass reads PSUM with the unpermuting
    # strided AP (strided PSUM reads are full speed on ACT; strided SBUF
    # reads/writes are ~2x slower, so everything else stays contiguous).
    # The three ACT passes run in place in one tile so the work pool gives
    # ~3 chunks of slack before DMA completions gate slot reuse.
    with (
        tc.tile_pool(name="qkps", bufs=1, space="PSUM") as qkpsum,
        tc.tile_pool(name="ctxps", bufs=2, space="PSUM") as ctxpsum,
        tc.tile_pool(name="outps", bufs=1, space="PSUM") as outpsum,
        tc.tile_pool(name="work", bufs=6) as work,
        tc.tile_pool(name="osbp", bufs=2) as osbp,
        tc.tile_pool(name="workh", bufs=3) as workh,
        tc.tile_pool(name="ctxsb", bufs=2) as ctxsb,
        tc.tile_pool(name="small", bufs=6) as small,
    ):
        def emit_ctx_group(p, qn, ctxT):
            # ctx.T = sum_k vp[k, d] * att_T[k, q~] for q~ in [qn*512, +512)
            # (needs only the transposes of q-chunks 4qn..4qn+3)
            ps_ctx = ctxpsum.tile([64, 512], F32, tag="ctx")
            for kc in range(16):
                nc.tensor.matmul(
                    ps_ctx[:],
                    vp[:, p * 1024 + kc * 64: p * 1024 + (kc + 1) * 64],
                    attT[qn][:, kc * 512:(kc + 1) * 512],
                    start=(kc == 0), stop=(kc == 15),
                )
            for i in range(4):
                s_lo = 4 * qn + i
                nc.vector.tensor_copy(
                    ctxT[(s_lo % 2) * 64:(s_lo % 2) * 64 + 64,
                         (s_lo // 2) * 128:(s_lo // 2) * 128 + 128],
                    ps_ctx[:, i * 128:(i + 1) * 128],
                )

        for p in range(HPC):
            ctxT = ctxsb.tile([128, 8 * 128], F16, tag="ctxT")
            for qc in range(16):
                if qc % 4 == 3 and qc > 3:
                    # ctx group whose transposes completed a few chunks ago.
                    # Delay ~6 chunks of priority so these matmuls are
                    # certainly ready when the tensor FIFO reaches them
                    # (a not-ready matmul head-of-line-blocks later qk work).
                    with tc.high_priority(offset=-120):
                        emit_ctx_group(p, qc // 4 - 1, ctxT)
                lhs = QhT[:, p * S + qc * 128: p * S + (qc + 1) * 128]
                ps_qk = qkpsum.tile([128, S], F32, tag="qk")
                for kn in range(4):
                    nc.tensor.matmul(
                        ps_qk[:, kn * 512:(kn + 1) * 512], lhs,
                        KhT[:, p * S + kn * 512: p * S + (kn + 1) * 512],
                        start=True, stop=True,
                    )
                # pass 1: t = ln(qk), unpermuting k~ = (sl, r) -> k = (r, sl)
                t = work.tile([128, S], F32, tag="wf")
                nc.scalar.activation(
                    t[:].rearrange("p (r sl) -> p r sl", sl=16),
                    ps_qk[:].rearrange("p (sl r) -> p r sl", sl=16),
                    AF.Ln,
                )
                # pass 2 (in place): r = exp(0.5 t) = sqrt(qk)
                nc.scalar.activation(t[:], t[:], AF.Exp, scale=0.5)
                # pass 3 (in place): e = exp(r - 40), row sums into dsum
                dsum = small.tile([128, 1], F32, tag="ds")
                nc.scalar.activation(
                    t[:], t[:], AF.Exp, bias=bias_t[:], accum_out=dsum[:])
                rec = small.tile([128, 1], F32, tag="rc")
                nc.vector.reciprocal(rec[:], dsum[:])
                att_f = work.tile([128, S], F32, tag="wf")
                nc.vector.tensor_scalar_mul(att_f[:], t[:], rec[:])
                nc.gpsimd.dma_start(
                    att_o[p].rearrange("(r sl) k -> sl r k", sl=16)[qc],
                    att_f[:],
                )
                att_h = workh.tile([128, S], F16, tag="wh")
                nc.vector.tensor_scalar_mul(att_h[:], t[:], rec[:])
                dst = attT[qc // 4][:, :].rearrange(
                    "p (kc q) -> p kc q",
                    kc=16)[:, :, (qc % 4) * 128:(qc % 4) * 128 + 128]
                nc.sync.dma_start_transpose(dst, att_h[:])

            with tc.high_priority(offset=-110):
                emit_ctx_group(p, 3, ctxT)
                ps_out = outpsum.tile([128, D], F32, tag="out")
                for j in range(8):
                    for mn in range(2):
                        nc.tensor.matmul(
                            ps_out[:, mn * 512:(mn + 1) * 512],
                            ctxT[:, j * 128:(j + 1) * 128],
                            wfc[:, j * D + mn * 512: j * D + (mn + 1) * 512],
                            start=(j == 0), stop=(j == 7),
                        )
                out_sb = osbp.tile([128, D], F32, tag="osb")
                nc.vector.tensor_copy(out_sb[:], ps_out[:])
                nc.gpsimd.dma_start(out_o[p * 128:(p + 1) * 128, :], out_sb[:])

    persist_cm.__exit__(None, None, None)


_NC_CACHE = None


def _get_nc():
    global _NC_CACHE
    if _NC_CACHE is None:
        _NC_CACHE = _build()
    return _NC_CACHE


def _prep_inputs(q, k, v, Wq, Wk, Wv, Wfc):
    f16 = ml_dtypes.float16 if hasattr(ml_dtypes, "float16") else np.float16
    WqT = np.ascontiguousarray(Wq.T).astype(f16)
    WkT = np.ascontiguousarray(Wk.T).astype(f16)
    WvT = np.ascontiguousarray(Wv.T).astype(f16)
    WfcT = np.ascontiguousarray(Wfc.T).astype(f16)
    in_maps = []
    for c in range(NC):
        b, g = c // (NC // B), c % (NC // B)
        rows = slice(ROWS * g, ROWS * (g + 1))
        in_maps.append({
            "qT": np.ascontiguousarray(q[b, rows, :].T).astype(f16),
            "kT": np.ascontiguousarray(k[b, rows, :].T).astype(f16),
            "vT": np.ascontiguousarray(v[b, rows, :].T).astype(f16),
            "WqT": WqT, "WkT": WkT, "WvT": WvT, "WfcT": WfcT,
        })
    return in_maps


def kernel(v, k, q, msk, Wq, Wk, Wv, Wfc, _trace=False, _trace_kwargs=None):
    q = np.asarray(q, np.float32)
    k = np.asarray(k, np.float32)
    v = np.asarray(v, np.float32)
    # msk is all-zeros per the problem spec (fill: zeros); msk * -1e9 == 0.
    nc = _get_nc()
    in_maps = _prep_inputs(q, k, v, np.asarray(Wq), np.asarray(Wk),
                           np.asarray(Wv), np.asarray(Wfc))
    kw = dict(_trace_kwargs or {})
    res = bass_utils.run_bass_kernel_spmd(
        nc, in_maps, core_ids=list(range(NC)), trace=_trace, **kw)
    att = np.empty((B, H, S, S), np.float32)
    out = np.empty((B, S, D), np.float32)
    for c in range(NC):
        b, g = c // (NC // B), c % (NC // B)
        r = res.results[c]
        att[b, HPC * g: HPC * (g + 1)] = r["att_o"]
        out[b, ROWS * g: ROWS * (g + 1), :] = r["out_o"]
    kernel.last_results = res
    return out, att
